# revision 12
# baseline (speedup 1.0000x reference)
"""v5 experiment: bf16-cast SWDGE stores (see kernel.py docstring for base)."""

import numpy as np

import concourse.bass as bass
import concourse.mybir as mybir
from concourse.bass_utils import run_bass_kernel_spmd

BATCH = 8192
SIZE = 4096
N_CORES = 8
ROWS = BATCH // N_CORES  # 1024
P = 128
NT = 16           # tiles: row-block i//2, col-block i%2
CB = SIZE // 2    # 2048
XB = 8            # xt ring depth

_CACHE: dict = {}


def _build() -> bass.Bass:
    nc = bass.Bass("TRN2", enable_asserts=False)
    f32 = mybir.dt.float32
    bf16 = mybir.dt.bfloat16
    x = nc.dram_tensor("x", [ROWS, SIZE], f32, kind="ExternalInput")
    dg = nc.dram_tensor("diagonal", [SIZE], f32, kind="ExternalInput")
    out = nc.dram_tensor("out", [ROWS, SIZE], f32, kind="ExternalOutput")

    xt = [nc.alloc_sbuf_tensor(f"xt{b}", [P, CB], f32) for b in range(XB)]
    yt = [nc.alloc_sbuf_tensor(f"yt{i}", [P, CB], bf16) for i in range(NT)]
    dtile = nc.alloc_sbuf_tensor("dtile", [P, SIZE], f32)
    warm = nc.alloc_sbuf_tensor("warm", [1, P], f32)

    def rs(i):
        r = (i // 2) * P
        return slice(r, r + P)

    def cs(i):
        c = (i % 2) * CB
        return slice(c, c + CB)

    from contextlib import ExitStack

    with ExitStack() as es, nc.Block(no_gpsimd_drain=True) as block:
        sem_dg = [es.enter_context(nc.semaphore(f"sem_dg{h}")) for h in range(2)]
        sem_mul = es.enter_context(nc.semaphore("sem_mul"))
        sem_st = es.enter_context(nc.semaphore("sem_st"))
        sem_warm = es.enter_context(nc.semaphore("sem_warm"))
        sem_ld = [es.enter_context(nc.semaphore(f"sem_ld{i}")) for i in range(NT)]

        def load(eng, i):
            if i >= XB:
                eng.wait_ge(sem_mul, i - XB + 1)
            eng.dma_start(out=xt[i % XB].ap(), in_=x[rs(i), cs(i)]).then_inc(
                sem_ld[i], 16
            )

        @block.sync
        def _(sp):
            for h in range(2):
                sp.dma_start(
                    out=dtile.ap()[:, h * CB : (h + 1) * CB],
                    in_=dg[h * CB : (h + 1) * CB].partition_broadcast(P),
                ).then_inc(sem_dg[h], 16)
            for i in range(0, NT, 2):
                load(sp, i)
            sp.wait_ge(sem_st, 16 * NT)

        @block.scalar
        def _(act):
            for i in range(1, NT, 2):
                load(act, i)

        @block.vector
        def _(dve):
            for i in range(NT):
                if i < 2:
                    dve.wait_ge(sem_dg[i % 2], 16)
                dve.wait_ge(sem_ld[i], 16)
                dve.tensor_mul(
                    yt[i].ap(), xt[i % XB].ap(), dtile.ap()[:, cs(i)]
                ).then_inc(sem_mul, 1)

        @block.gpsimd
        def _(gp):
            gp.dma_start(out=warm.ap(), in_=dg[0:P]).then_inc(sem_warm, 16)
            gp.wait_ge(sem_warm, 16)
            for i in range(NT):
                gp.wait_ge(sem_mul, i + 1)
                gp.dma_start(out=out[rs(i), cs(i)], in_=yt[i].ap()).then_inc(
                    sem_st, 16
                )

    blocks = nc.m.functions[0].blocks
    blocks[0].instructions = [
        inst
        for inst in blocks[0].instructions
        if type(inst).__name__ not in ("InstDrain", "InstEventSemaphore", "InstMemset")
    ]
    end_bb = blocks[-1]
    end_bb.instructions = [
        inst
        for inst in end_bb.instructions
        if type(inst).__name__ not in ("InstDrain", "InstEventSemaphore")
    ]
    return nc


def kernel(x: np.ndarray, diagonal: np.ndarray) -> np.ndarray:
    if "nc" not in _CACHE:
        _CACHE["nc"] = _build()
    nc = _CACHE["nc"]

    x = np.ascontiguousarray(np.asarray(x, dtype=np.float32))
    diagonal = np.ascontiguousarray(np.asarray(diagonal, dtype=np.float32))

    shards = np.split(x, N_CORES, axis=0)
    in_maps = [{"x": s, "diagonal": diagonal} for s in shards]
    res = run_bass_kernel_spmd(nc, in_maps, list(range(N_CORES))).results
    return np.concatenate([r["out"] for r in res], axis=0)
